# revision 6
# baseline (speedup 1.0000x reference)
"""MQA self-attention kernel for Trainium2, 8 NeuronCores.

Reference computation (fp32):
    q = x @ wq.T + bq        -> [B,S,1024] -> heads via (hidden num_heads) split
    k = x @ wk.T + bk        -> [B,S,64]  (single shared KV head)
    v = x @ wv.T + bv
    scores = q @ k.T / 8 ; attn = softmax(scores) ; h = attn @ v
    out = merge_heads(h) @ wo.T + bo

Sharding (8 cores, no collectives): core c handles batch b=c//4 and head
group g=c%4 (4 of the 16 q-heads).  The shared K/V head is replicated.
Each core returns the partial output h_g @ wo_g.T [S, D]; the host sums
the 4 head-group partials per batch and adds the bias terms.

Math notes:
 - bk provably cancels in softmax (adds a per-row constant to scores).
 - bv is folded into the output bias on host: softmax rows sum to 1, so
   attn @ (v + bv) = attn @ v + bv, contributing wo @ tile(bv, 16).
 - softmax is computed without max subtraction (scores ~ N(0,1); exp is
   safe in fp32) which lets exp(scores) @ [V|1] accumulate both the
   numerator and denominator in one PSUM pass.

Device layout (everything transposed so no on-device input transposes):
 - xT [1024, 2048] (d on partitions) is the rhs for all projections.
 - QT[q,s], KT[d,s] computed directly in transposed layout.
 - scoresT[sk,sq] = KT.T @ QT per head, 2 heads per pass via 64x128
   row-tiled matmuls (contraction is only Dh=64).
 - exp on ScalarE (PSUM -> SBUF), PV matmul with V'=[V|1] (M=65) gives
   hT_un[d,sq] plus the softmax denominator in the same accumulation.
 - normalize via DVE reciprocal + broadcast-DMA + DVE multiply.
 - out partial = hT.T @ woT via PSUM accumulation over the 256 q dims.
All matmuls run in float32r (TF32-like, full PE rate at N=512).
"""

import numpy as np

NUM_HEADS = 16
Dh = 64
B, S, D = 2, 2048, 1024
G = 4            # head groups (cores per batch)
HG = 4           # heads per group
QD = HG * Dh     # 256 local q dims
NK = D // 128    # 8 contraction tiles for projections
NSK = S // 128   # 16 key tiles
W = 512          # sq quarter width
NW = S // W      # 4 quarters
N_CORES = 8

_CACHE = {}


def _build_nc():
    from contextlib import ExitStack

    import concourse.bass as bass
    import concourse.mybir as mybir
    import concourse.tile as tile
    from concourse import bacc
    from concourse.masks import make_identity

    F32 = mybir.dt.float32
    F32R = mybir.dt.float32r
    EXP = mybir.ActivationFunctionType.Exp

    def r(ap):
        return ap

    nc = bacc.Bacc("TRN2", target_bir_lowering=False, debug=False)

    xT = nc.declare_dram_parameter("xT", [D, S], F32R, isOutput=False)
    wqT = nc.declare_dram_parameter("wqT", [D, QD], F32R, isOutput=False)
    wvkT = nc.declare_dram_parameter("wvkT", [D, 128], F32R, isOutput=False)
    woT = nc.declare_dram_parameter("woT", [QD, D], F32R, isOutput=False)
    bqp = nc.declare_dram_parameter("bq", [QD, 1], F32, isOutput=False)
    part = nc.declare_dram_parameter("part", [S, D], F32, isOutput=True)

    with tile.TileContext(nc) as tc, ExitStack() as ctx:
        const = ctx.enter_context(tc.tile_pool(name="const", bufs=1))
        persist = ctx.enter_context(tc.tile_pool(name="persist", bufs=1))

        wq_sb = const.tile([128, NK * QD], F32R)    # ktile kt at cols [kt*QD:+QD]
        wvk_sb = const.tile([128, NK * 128], F32R)  # cols 0:64 of each ktile = wvT, 64:128 = wkT
        wo_sb = const.tile([128, 2 * D], F32R)      # q-ktile p at cols [p*D:+D]
        bq_sb = const.tile([128, 2], F32)
        ident = const.tile([128, 128], F32)
        ones_sb = const.tile([128, 1], F32)

        qt_sb = persist.tile([128, 2 * S], F32R)    # q ptile p (pair p) at cols [p*S:+S]
        ktd_sb = persist.tile([128, S], F32R)       # KT duplicated: rows 0:64 == rows 64:128
        v1_sb = persist.tile([128, NSK * 65], F32R)  # V' tile sk at cols [sk*65:+65]
        ht_sb = persist.tile([128, 2 * S], F32R)    # normalized hT, ptile p at cols [p*S:+S]

        for p in range(2):
            nc.sync.dma_start(wo_sb[:, p * D:(p + 1) * D], woT[p * 128:(p + 1) * 128, :])
            nc.sync.dma_start(bq_sb[:, p:p + 1], bqp[p * 128:(p + 1) * 128, :])
        make_identity(nc, ident[:])
        nc.vector.memset(ones_sb[:], 1.0)

        # ---- Phase 1: projections -------------------------------------
        with tc.tile_pool(name="xp", bufs=1) as xp:
            x_sb = xp.tile([128, NK * S], F32R)     # xT ktile kt at cols [kt*S:+S]
            vt_sb = xp.tile([128, S], F32)         # VT in rows 0:64
            for kt in range(NK):
                nc.sync.dma_start(x_sb[:, kt * S:(kt + 1) * S], xT[kt * 128:(kt + 1) * 128, :])
                nc.sync.dma_start(wq_sb[:, kt * QD:(kt + 1) * QD], wqT[kt * 128:(kt + 1) * 128, :])
                nc.sync.dma_start(wvk_sb[:, kt * 128:(kt + 1) * 128], wvkT[kt * 128:(kt + 1) * 128, :])

            # fused [V|K] projection: VKT[0:64]=VT, VKT[64:128]=KT
            with tc.tile_pool(name="vkps", bufs=1, space="PSUM") as vkps:
                vk_ps = vkps.tile([128, S], F32)
                for kt in range(NK):
                    for n in range(S // W):
                        nc.tensor.matmul(
                            vk_ps[:, n * W:(n + 1) * W],
                            lhsT=r(wvk_sb)[:, kt * 128:(kt + 1) * 128],
                            rhs=r(x_sb)[:, kt * S + n * W: kt * S + (n + 1) * W],
                            start=(kt == 0), stop=(kt == NK - 1),
                        )
                nc.vector.tensor_copy(vt_sb[0:64, :], vk_ps[0:64, :])
                nc.vector.tensor_copy(ktd_sb[64:128, :], vk_ps[64:128, :])
            # duplicate KT into rows 0:64 (re-partitioning SBUF->SBUF DMA)
            nc.sync.dma_start(ktd_sb[0:64, :], ktd_sb[64:128, :])

            # V' tiles: PE-transpose VT -> V[sk] = [128, 64], plus ones column
            with tc.tile_pool(name="trps", bufs=2, space="PSUM") as trps:
                for sk in range(NSK):
                    tr_ps = trps.tile([128, Dh], F32)
                    nc.tensor.transpose(
                        tr_ps[:], vt_sb[0:64, sk * 128:(sk + 1) * 128], ident[0:64, 0:64]
                    )
                    nc.vector.tensor_copy(v1_sb[:, sk * 65: sk * 65 + 64], tr_ps[:])
                    nc.vector.tensor_copy(v1_sb[:, sk * 65 + 64: sk * 65 + 65], ones_sb[:])

            # QT projection (2 partition tiles of 128 q rows) + bias
            with tc.tile_pool(name="qps", bufs=2, space="PSUM") as qps:
                for m in range(2):
                    q_ps = qps.tile([128, S], F32)
                    for kt in range(NK):
                        for n in range(S // W):
                            nc.tensor.matmul(
                                q_ps[:, n * W:(n + 1) * W],
                                lhsT=r(wq_sb)[:, kt * QD + m * 128: kt * QD + (m + 1) * 128],
                                rhs=r(x_sb)[:, kt * S + n * W: kt * S + (n + 1) * W],
                                start=(kt == 0), stop=(kt == NK - 1),
                            )
                    nc.vector.tensor_scalar_add(
                        qt_sb[:, m * S:(m + 1) * S], q_ps[:], bq_sb[:, m:m + 1]
                    )

        # ---- Phase 2/3: attention + output projection ------------------
        with (
            tc.tile_pool(name="expp", bufs=1) as expp,
            tc.tile_pool(name="scps", bufs=2, space="PSUM") as scps,
            tc.tile_pool(name="pvps", bufs=1, space="PSUM") as pvps,
            tc.tile_pool(name="outps", bufs=2, space="PSUM") as outps,
            tc.tile_pool(name="smalls", bufs=4) as smalls,
            tc.tile_pool(name="bcp", bufs=4) as bcp,
            tc.tile_pool(name="osbp", bufs=3) as osbp,
        ):
            expA_sb = expp.tile([128, NSK * W], F32R)
            expB_sb = expp.tile([128, NSK * W], F32R)

            for w in range(NW):
                for p in range(2):
                    qcol = p * S + w * W
                    # scores^T for the head pair, 64x128 row-tiled
                    for sk in range(NSK):
                        sA = scps.tile([128, W], F32, name="sA")
                        sB = scps.tile([128, W], F32, name="sB")
                        nc.tensor.matmul(
                            sA[:],
                            lhsT=r(ktd_sb)[0:64, sk * 128:(sk + 1) * 128],
                            rhs=r(qt_sb)[0:64, qcol:qcol + W],
                            start=True, stop=True, tile_position=(0, 0),
                        )
                        nc.tensor.matmul(
                            sB[:],
                            lhsT=r(ktd_sb)[64:128, sk * 128:(sk + 1) * 128],
                            rhs=r(qt_sb)[64:128, qcol:qcol + W],
                            start=True, stop=True, tile_position=(64, 0),
                        )
                        nc.scalar.activation(
                            expA_sb[:, sk * W:(sk + 1) * W], sA[:], EXP, scale=0.125
                        )
                        nc.scalar.activation(
                            expB_sb[:, sk * W:(sk + 1) * W], sB[:], EXP, scale=0.125
                        )

                    # hT' = [V|1].T @ expT : rows 0:64 = hT_un, row 64 = sumexp
                    hA = pvps.tile([128, W], F32, name="hA")
                    hB = pvps.tile([128, W], F32, name="hB")
                    for sk in range(NSK):
                        nc.tensor.matmul(
                            hA[0:65, :],
                            lhsT=r(v1_sb)[:, sk * 65:(sk + 1) * 65],
                            rhs=r(expA_sb)[:, sk * W:(sk + 1) * W],
                            start=(sk == 0), stop=(sk == NSK - 1),
                        )
                        nc.tensor.matmul(
                            hB[0:65, :],
                            lhsT=r(v1_sb)[:, sk * 65:(sk + 1) * 65],
                            rhs=r(expB_sb)[:, sk * W:(sk + 1) * W],
                            start=(sk == 0), stop=(sk == NSK - 1),
                        )

                    # normalize: hT[:, sq] /= sumexp[sq]
                    recA = smalls.tile([128, W], F32, name="recA")
                    recB = smalls.tile([128, W], F32, name="recB")
                    nc.vector.reciprocal(recA[64:65, :], hA[64:65, :])
                    nc.vector.reciprocal(recB[64:65, :], hB[64:65, :])
                    bcA = bcp.tile([128, W], F32, name="bcA")
                    bcB = bcp.tile([128, W], F32, name="bcB")
                    sA_ = recA[64:65, :]
                    sB_ = recB[64:65, :]
                    nc.sync.dma_start(
                        bcA[0:64, :],
                        bass.AP(sA_.tensor, sA_.offset, [list(sA_.ap[0]), [0, 64], [1, W]]),
                    )
                    nc.sync.dma_start(
                        bcB[0:64, :],
                        bass.AP(sB_.tensor, sB_.offset, [list(sB_.ap[0]), [0, 64], [1, W]]),
                    )
                    nc.vector.tensor_mul(
                        ht_sb[0:64, p * S + w * W: p * S + (w + 1) * W],
                        hA[0:64, :], bcA[0:64, :],
                    )
                    tmpB = bcp.tile([128, W], F32R, name="tmpB")
                    nc.vector.tensor_mul(tmpB[0:64, :], hB[0:64, :], bcB[0:64, :])
                    # head B lives in ht rows 64:128 -> re-partition DMA
                    nc.sync.dma_start(
                        ht_sb[64:128, p * S + w * W: p * S + (w + 1) * W], tmpB[0:64, :]
                    )

                # output projection for the 4 s-chunks of this quarter
                for sc in range(4):
                    s = w * 4 + sc
                    for n in range(2):
                        o_ps = outps.tile([128, W], F32, name="ops")
                        for p in range(2):
                            nc.tensor.matmul(
                                o_ps[:],
                                lhsT=r(ht_sb)[:, p * S + s * 128: p * S + (s + 1) * 128],
                                rhs=r(wo_sb)[:, p * D + n * W: p * D + (n + 1) * W],
                                start=(p == 0), stop=(p == 1),
                            )
                        o_sb = osbp.tile([128, W], F32, name="osb")
                        nc.vector.tensor_copy(o_sb[:], o_ps[:])
                        nc.sync.dma_start(
                            part[s * 128:(s + 1) * 128, n * W:(n + 1) * W], o_sb[:]
                        )

    nc.finalize()
    return nc


def _get_nc():
    if "nc" not in _CACHE:
        _CACHE["nc"] = _build_nc()
    return _CACHE["nc"]


def _prep_core_inputs(inputs, wq, bq, wk, wv, wo):
    """Host-side shard prep: per-core transposed/rearranged operands."""
    xT = [np.ascontiguousarray(np.asarray(inputs[b], np.float32).T) for b in range(B)]
    wq3 = np.asarray(wq, np.float32).reshape(Dh, NUM_HEADS, D)
    bq2 = np.asarray(bq, np.float32).reshape(Dh, NUM_HEADS)
    wvkT = np.ascontiguousarray(
        np.concatenate([np.asarray(wv, np.float32).T, np.asarray(wk, np.float32).T], axis=1)
    )  # [1024, 128]
    wo_ = np.asarray(wo, np.float32)

    in_maps = []
    for c in range(N_CORES):
        b, g = divmod(c, G)
        heads = [g * HG + hl for hl in range(HG)]
        # wqT_g [1024, 256]: column block hl = head (g*HG+hl), rows = d
        wqT_g = np.ascontiguousarray(
            np.concatenate([wq3[:, h, :].T for h in heads], axis=1)
        )
        bq_g = np.ascontiguousarray(
            np.concatenate([bq2[:, h] for h in heads]).reshape(QD, 1)
        )
        woT_g = np.ascontiguousarray(wo_[:, g * QD:(g + 1) * QD].T)  # [256, 1024]
        in_maps.append({
            "xT": xT[b],
            "wqT": wqT_g,
            "wvkT": wvkT,
            "woT": woT_g,
            "bq": bq_g,
        })
    return in_maps


def kernel(inputs, wq, bq, wk, bk, wv, bv, wo, bo):
    from concourse.bass_utils import run_bass_kernel_spmd

    nc = _get_nc()
    in_maps = _prep_core_inputs(inputs, wq, bq, wk, wv, wo)
    res = run_bass_kernel_spmd(nc, in_maps, list(range(N_CORES))).results

    wo_ = np.asarray(wo, np.float32)
    bias = (
        np.asarray(bo, np.float32)
        + wo_ @ np.tile(np.asarray(bv, np.float32), NUM_HEADS)
    )
    out = np.empty((B, S, D), np.float32)
    for b in range(B):
        acc = res[b * G]["part"].astype(np.float32).copy()
        for g in range(1, G):
            acc += res[b * G + g]["part"]
        out[b] = acc + bias
    return out


# revision 13
# speedup vs baseline: 37.2021x; 37.2021x over previous
"""MQA self-attention kernel for Trainium2, 8 NeuronCores.

Reference computation (fp32):
    q = x @ wq.T + bq        -> [B,S,1024] -> heads via (hidden num_heads) split
    k = x @ wk.T + bk        -> [B,S,64]  (single shared KV head)
    v = x @ wv.T + bv
    scores = q @ k.T / 8 ; attn = softmax(scores) ; h = attn @ v
    out = merge_heads(h) @ wo.T + bo

Sharding (8 cores, no collectives): core c handles batch b=c//4 and head
group g=c%4 (4 of the 16 q-heads).  The shared K/V head is replicated.
Each core returns the partial output h_g @ wo_g.T [S, D]; the host sums
the 4 head-group partials per batch and adds the bias terms.

Math notes:
 - bk provably cancels in softmax (adds a per-row constant to scores).
 - bv is folded into the output bias on host: softmax rows sum to 1, so
   attn @ (v + bv) = attn @ v + bv, contributing wo @ tile(bv, 16).
 - softmax is computed without max subtraction (scores ~ N(0,1); exp is
   safe in fp32) which lets exp(scores) @ [V|1] accumulate both the
   numerator and denominator in one PSUM pass.

Device layout (everything transposed so no on-device input transposes):
 - xT [1024, 2048] (d on partitions) is the rhs for all projections.
 - QT[q,s] and KT[d,s] computed directly in transposed layout; both are
   duplicated across SBUF partition halves so the scores matmul (K=64)
   runs as 64x128 row-tiled pairs over two key tiles at once.
 - scoresT[sk,sq] = KT.T @ QT per head; exp on ScalarE in [128,1024]
   blocks (amortizes the per-ACTIVATE overhead), PV matmul with V'=[V|1]
   (M=65) gives hT_un[d,sq] plus the softmax denominator in the same
   PSUM accumulation.
 - normalize via DVE reciprocal + broadcast-DMA + DVE multiply.
 - out partial = hT.T @ woT via PSUM accumulation over the 256 q dims.
All matmuls run in float32r (TF32-like, full PE rate at N=512).
"""

import numpy as np

NUM_HEADS = 16
Dh = 64
B, S, D = 2, 2048, 1024
G = 4            # head groups (cores per batch)
HG = 4           # heads per group
QD = HG * Dh     # 256 local q dims
NK = D // 128    # 8 contraction tiles for projections
NSK = S // 128   # 16 key tiles
W = 512          # matmul moving width
BLK = 1024       # sq block width for exp
NB = S // BLK    # 2 blocks
N_CORES = 8

_CACHE = {}


def _build_nc():
    from contextlib import ExitStack

    import concourse.bass as bass
    import concourse.mybir as mybir
    import concourse.tile as tile
    from concourse import bacc
    from concourse.masks import make_identity

    F32 = mybir.dt.float32
    F32R = mybir.dt.float32r
    EXP = mybir.ActivationFunctionType.Exp

    nc = bacc.Bacc("TRN2", target_bir_lowering=False, debug=False)

    xT = nc.declare_dram_parameter("xT", [D, S], F32R, isOutput=False)
    wqT = nc.declare_dram_parameter("wqT", [D, QD], F32R, isOutput=False)
    wvkT = nc.declare_dram_parameter("wvkT", [D, 128], F32R, isOutput=False)
    woT = nc.declare_dram_parameter("woT", [QD, D], F32R, isOutput=False)
    bqp = nc.declare_dram_parameter("bq", [QD, 1], F32, isOutput=False)
    part = nc.declare_dram_parameter("part", [S, D], F32, isOutput=True)

    with tile.TileContext(nc) as tc, ExitStack() as ctx:
        const = ctx.enter_context(tc.tile_pool(name="const", bufs=1))
        persist = ctx.enter_context(tc.tile_pool(name="persist", bufs=1))

        wq_sb = const.tile([128, NK * QD], F32R)    # ktile kt at cols [kt*QD:+QD]
        wvk_sb = const.tile([128, NK * 128], F32R)  # cols 0:64 of each ktile = wvT, 64:128 = wkT
        wo_sb = const.tile([128, 2 * D], F32R)      # q-ktile p at cols [p*D:+D]
        bq_sb = const.tile([128, 2], F32)
        ident = const.tile([128, 128], F32)
        ones_sb = const.tile([128, 1], F32)

        # qtd: per-head QT duplicated across both partition halves:
        # head h at cols [h*S:+S], rows 0:64 == rows 64:128 == QT_h [64, S]
        qtd_sb = persist.tile([128, HG * S], F32R)
        ktd_sb = persist.tile([128, S], F32R)       # KT duplicated: rows 0:64 == rows 64:128
        v1_sb = persist.tile([128, NSK * 65], F32R)  # V' tile sk at cols [sk*65:+65]
        ht_sb = persist.tile([128, 2 * S], F32R)    # normalized hT, q-ktile p at cols [p*S:+S]

        make_identity(nc, ident[:])
        nc.vector.memset(ones_sb[:], 1.0)

        # ---- Phase 1: projections -------------------------------------
        with tc.tile_pool(name="xp", bufs=1) as xp:
            x_sb = xp.tile([128, NK * S], F32R)     # xT ktile kt at cols [kt*S:+S]
            vt_sb = xp.tile([128, S], F32)          # VT in rows 0:64
            for kt in range(NK):
                nc.sync.dma_start(wvk_sb[:, kt * 128:(kt + 1) * 128], wvkT[kt * 128:(kt + 1) * 128, :])
            for kt in range(NK):
                nc.sync.dma_start(wq_sb[:, kt * QD:(kt + 1) * QD], wqT[kt * 128:(kt + 1) * 128, :])
            for kt in range(NK):
                nc.sync.dma_start(x_sb[:, kt * S:(kt + 1) * S], xT[kt * 128:(kt + 1) * 128, :])
            for p in range(2):
                nc.sync.dma_start(bq_sb[:, p:p + 1], bqp[p * 128:(p + 1) * 128, :])
                nc.sync.dma_start(wo_sb[:, p * D:(p + 1) * D], woT[p * 128:(p + 1) * 128, :])

            # fused [V|K] projection interleaved with QT heads 0/1 so both
            # finish right as the last x tile lands.
            with (
                tc.tile_pool(name="vkps", bufs=1, space="PSUM") as vkps,
                tc.tile_pool(name="qps", bufs=1, space="PSUM") as qps,
            ):
                vk_ps = vkps.tile([128, S], F32)
                q_ps = qps.tile([128, S], F32)
                for kt in range(NK):
                    for n in range(S // W):
                        nc.tensor.matmul(
                            vk_ps[:, n * W:(n + 1) * W],
                            lhsT=wvk_sb[:, kt * 128:(kt + 1) * 128],
                            rhs=x_sb[:, kt * S + n * W: kt * S + (n + 1) * W],
                            start=(kt == 0), stop=(kt == NK - 1),
                        )
                    for n in range(S // W):
                        nc.tensor.matmul(
                            q_ps[:, n * W:(n + 1) * W],
                            lhsT=wq_sb[:, kt * QD: kt * QD + 128],
                            rhs=x_sb[:, kt * S + n * W: kt * S + (n + 1) * W],
                            start=(kt == 0), stop=(kt == NK - 1),
                        )
                # evacs split across DVE and the (still idle) ScalarE so the
                # path to the first scores matmul is short
                nc.vector.tensor_scalar_add(
                    qtd_sb[0:64, 0:S], q_ps[0:64, :], bq_sb[0:64, 0:1]
                )
                nc.scalar.copy(ktd_sb[64:128, :], vk_ps[64:128, :])
                nc.sync.dma_start(ktd_sb[0:64, :], ktd_sb[64:128, :])
                nc.scalar.copy(vt_sb[0:64, :], vk_ps[0:64, :])
                nc.vector.tensor_scalar_add(
                    qtd_sb[64:128, S:2 * S], q_ps[64:128, :], bq_sb[64:128, 0:1]
                )
                nc.sync.dma_start(qtd_sb[0:64, S:2 * S], qtd_sb[64:128, S:2 * S])

            # QT heads 2/3 (reuses the freed PSUM banks)
            with tc.tile_pool(name="qps2", bufs=1, space="PSUM") as qps2:
                q_ps2 = qps2.tile([128, S], F32)
                for kt in range(NK):
                    for n in range(S // W):
                        nc.tensor.matmul(
                            q_ps2[:, n * W:(n + 1) * W],
                            lhsT=wq_sb[:, kt * QD + 128: kt * QD + 256],
                            rhs=x_sb[:, kt * S + n * W: kt * S + (n + 1) * W],
                            start=(kt == 0), stop=(kt == NK - 1),
                        )
                nc.vector.tensor_scalar_add(
                    qtd_sb[0:64, 2 * S:3 * S], q_ps2[0:64, :], bq_sb[0:64, 1:2]
                )
                nc.vector.tensor_scalar_add(
                    qtd_sb[64:128, 3 * S:4 * S], q_ps2[64:128, :], bq_sb[64:128, 1:2]
                )
                nc.sync.dma_start(qtd_sb[0:64, 3 * S:4 * S], qtd_sb[64:128, 3 * S:4 * S])

            # V' tiles: PE-transpose VT -> V[sk] = [128, 64], plus ones column
            with tc.tile_pool(name="trps", bufs=2, space="PSUM") as trps:
                for sk in range(NSK):
                    tr_ps = trps.tile([128, Dh], F32)
                    nc.tensor.transpose(
                        tr_ps[:], vt_sb[0:64, sk * 128:(sk + 1) * 128], ident[0:64, 0:64]
                    )
                    nc.vector.tensor_copy(v1_sb[:, sk * 65: sk * 65 + 64], tr_ps[:])
                    nc.vector.tensor_copy(v1_sb[:, sk * 65 + 64: sk * 65 + 65], ones_sb[:])

        # ---- Phase 2/3: attention + output projection ------------------
        # Software-pipelined at key-tile granularity: scores(sk) -> exp(sk)
        # -> PV(sk-2), so ScalarE (the exp bottleneck) never idles.
        with (
            tc.tile_pool(name="expp", bufs=6) as expp,
            tc.tile_pool(name="scps", bufs=2, space="PSUM") as scps,
            tc.tile_pool(name="pvps", bufs=1, space="PSUM") as pvps,
            tc.tile_pool(name="outps", bufs=2, space="PSUM") as outps,
            tc.tile_pool(name="smalls", bufs=4) as smalls,
            tc.tile_pool(name="bcp", bufs=4) as bcp,
            tc.tile_pool(name="osbp", bufs=3) as osbp,
        ):
            for b in range(NB):
                for h in range(HG):
                    qcol = h * S + b * BLK
                    hv = [pvps.tile([128, W], F32, name=f"pv{half}")
                          for half in range(BLK // W)]
                    exp_tiles = [None] * NSK

                    def emit_pv(sk):
                        for half in range(BLK // W):
                            nc.tensor.matmul(
                                hv[half][0:65, :],
                                lhsT=v1_sb[:, sk * 65:(sk + 1) * 65],
                                rhs=exp_tiles[sk][:, half * W:(half + 1) * W],
                                start=(sk == 0), stop=(sk == NSK - 1),
                            )

                    for sk in range(NSK):
                        if sk >= 2:
                            emit_pv(sk - 2)
                        sc = scps.tile([128, BLK], F32, name="sc")
                        for n in range(BLK // W):
                            nc.tensor.matmul(
                                sc[:, n * W:(n + 1) * W],
                                lhsT=ktd_sb[0:64, sk * 128:(sk + 1) * 128],
                                rhs=qtd_sb[0:64, qcol + n * W: qcol + (n + 1) * W],
                                start=True, stop=True,
                            )
                        et = expp.tile([128, BLK], F32R, name="expt")
                        nc.scalar.activation(et[:], sc[:], EXP, scale=0.125)
                        exp_tiles[sk] = et
                    emit_pv(NSK - 2)
                    emit_pv(NSK - 1)

                    # normalize: hT[:, sq] /= sumexp[sq]
                    for half in range(BLK // W):
                        rec = smalls.tile([128, W], F32, name="rec")
                        nc.vector.reciprocal(rec[64:65, :], hv[half][64:65, :])
                        bc = bcp.tile([128, W], F32, name="bc")
                        s_ = rec[64:65, :]
                        nc.sync.dma_start(
                            bc[0:64, :],
                            bass.AP(s_.tensor, s_.offset, [list(s_.ap[0]), [0, 64], [1, W]]),
                        )
                        hcol = (h // 2) * S + b * BLK + half * W
                        if h % 2 == 0:
                            nc.vector.tensor_mul(
                                ht_sb[0:64, hcol:hcol + W], hv[half][0:64, :], bc[0:64, :]
                            )
                        else:
                            tmp = bcp.tile([128, W], F32R, name="tmp")
                            nc.vector.tensor_mul(tmp[0:64, :], hv[half][0:64, :], bc[0:64, :])
                            # odd head lives in ht rows 64:128 -> re-partition DMA
                            nc.sync.dma_start(ht_sb[64:128, hcol:hcol + W], tmp[0:64, :])

                # output projection for the 8 s-chunks of this block
                for sc_i in range(BLK // 128):
                    s = b * (BLK // 128) + sc_i
                    for n in range(2):
                        o_ps = outps.tile([128, W], F32, name="ops")
                        for p in range(2):
                            nc.tensor.matmul(
                                o_ps[:],
                                lhsT=ht_sb[:, p * S + s * 128: p * S + (s + 1) * 128],
                                rhs=wo_sb[:, p * D + n * W: p * D + (n + 1) * W],
                                start=(p == 0), stop=(p == 1),
                            )
                        o_sb = osbp.tile([128, W], F32, name="osb")
                        nc.vector.tensor_copy(o_sb[:], o_ps[:])
                        nc.sync.dma_start(
                            part[s * 128:(s + 1) * 128, n * W:(n + 1) * W], o_sb[:]
                        )

    nc.finalize()
    return nc


def _get_nc():
    if "nc" not in _CACHE:
        _CACHE["nc"] = _build_nc()
    return _CACHE["nc"]


def _prep_core_inputs(inputs, wq, bq, wk, wv, wo):
    """Host-side shard prep: per-core transposed/rearranged operands."""
    xT = [np.ascontiguousarray(np.asarray(inputs[b], np.float32).T) for b in range(B)]
    wq3 = np.asarray(wq, np.float32).reshape(Dh, NUM_HEADS, D)
    bq2 = np.asarray(bq, np.float32).reshape(Dh, NUM_HEADS)
    wvkT = np.ascontiguousarray(
        np.concatenate([np.asarray(wv, np.float32).T, np.asarray(wk, np.float32).T], axis=1)
    )  # [1024, 128]
    wo_ = np.asarray(wo, np.float32)

    in_maps = []
    for c in range(N_CORES):
        b, g = divmod(c, G)
        heads = [g * HG + hl for hl in range(HG)]
        # wqT_g [1024, 256]: column block hl = head (g*HG+hl), rows = d
        wqT_g = np.ascontiguousarray(
            np.concatenate([wq3[:, h, :].T for h in heads], axis=1)
        )
        bq_g = np.ascontiguousarray(
            np.concatenate([bq2[:, h] for h in heads]).reshape(QD, 1)
        )
        woT_g = np.ascontiguousarray(wo_[:, g * QD:(g + 1) * QD].T)  # [256, 1024]
        in_maps.append({
            "xT": xT[b],
            "wqT": wqT_g,
            "wvkT": wvkT,
            "woT": woT_g,
            "bq": bq_g,
        })
    return in_maps


def kernel(inputs, wq, bq, wk, bk, wv, bv, wo, bo):
    from concourse.bass_utils import run_bass_kernel_spmd

    nc = _get_nc()
    in_maps = _prep_core_inputs(inputs, wq, bq, wk, wv, wo)
    res = run_bass_kernel_spmd(nc, in_maps, list(range(N_CORES))).results

    wo_ = np.asarray(wo, np.float32)
    bias = (
        np.asarray(bo, np.float32)
        + wo_ @ np.tile(np.asarray(bv, np.float32), NUM_HEADS)
    )
    out = np.empty((B, S, D), np.float32)
    for b in range(B):
        acc = res[b * G]["part"].astype(np.float32).copy()
        for g in range(1, G):
            acc += res[b * G + g]["part"]
        out[b] = acc + bias
    return out
